# revision 1
# baseline (speedup 1.0000x reference)
"""Stage-3 Trainium2 Bass kernel for nn_BilinearFullSymLoss.

Per-sample math (derivation in kernel2.py / kernel_v1.py):
  delta(i,j) = wA0*G(i,j) + wA1*G(i+1,j) + wB0*bc(i,j) + wB1*bc(i+1,j)
  bc(i,j)    = cb0*G(i+rb, j+cb) + cb1*G(i+rb, j+cb+1)   (col interp)
  pos: wA=(1,0),         wB=(-(1-fy),-fy), rb=dy1,   cb=dx1
  neg: wA=(-fy,-(1-fy)), wB=(1,0),         rb=dy1+1, cb=dx1
       (neg evaluated at j' = j-dx1; host sums columns [-dx1, W))
  loss = m^2 * sum(valid delta^2) / (rows*cols); host does the scalar math.

Device plan per core (4 samples):
- combine channels (ACT scale-copy || DVE tensor_scalar, then DVE add)
  -> G fp16 in SBUF, written back to a per-sample DRAM scratch region
- ONE dynamic-offset window win[p,q,:] = Gd[off + i*W .. +W+2) via sync
  HWDGE DMA (off loaded from SBUF into an SP register; this walrus build
  rejects runtime-assert and multi-wait instructions, hence
  skip_runtime_bounds_check and the _split_multiwaits post-pass)
- bc = col-interp of win (DVE tensor_scalar 4x fp16 + tensor_tensor 2x)
- delta accumulated in PSUM by PE matmuls with host-built banded lhsT
  (wA0*I + wA1*subdiag) @ G + (wB0*I + wB1*subdiag) @ bc, plus
  single-entry cross-seam matrices for the row-128 boundaries
- ACT Square(PSUM) -> sq fp16; PE matmul with a 0/1 ivalid stationary
  vector gives i-masked per-column sums; host sums the valid column range
"""

import sys

sys.path.insert(0, "/opt/trn_rl_repo")

import numpy as np

import concourse.bass as bass
import concourse.tile as tile
from concourse import mybir
from concourse.bass_utils import run_bass_kernel_spmd

H = 512
W = 512
P = 128
Q = H // P
NS = 4
NCORES = 8
RPAD = 16
REG = (H + RPAD) * W
WLEN = W + 2

F32 = mybir.dt.float32
F16 = mybir.dt.float16
I32 = mybir.dt.int32

NPF = 4   # a, b, cb0, cb1
COL_A, COL_B, COL_CB0, COL_CB1 = range(NPF)

_CACHE = {}


def _split_multiwaits(nc):
    """The staged walrus accepts one sync wait per instruction; hoist extras
    onto single-wait NoOps."""
    n = 0
    for fn in nc.m.functions:
        for bb in fn.blocks:
            newlist = []
            for ins in bb.instructions:
                si = ins.sync_info
                if si is not None and si.on_wait is not None and len(si.on_wait) > 1:
                    waits = list(si.on_wait)
                    for w in waits[:-1]:
                        n += 1
                        newlist.append(mybir.InstNoOp(
                            name=f"WSPLIT-{n}-{ins.name}", opcode="NoOp",
                            engine=ins.engine,
                            sync_info=mybir.SyncInfo(on_wait=[w], on_update=[])))
                    ins.sync_info = mybir.SyncInfo(
                        on_wait=[waits[-1]], on_update=list(si.on_update))
                newlist.append(ins)
            bb.instructions = newlist
    return n


def _grid_ap(g, s, c):
    return g[s, c].rearrange("(q p) w -> p q w", p=P)


def _flat_ap(gd, offset, inner):
    return bass.AP(tensor=gd, offset=offset, ap=[[W, P], [P * W, Q], [1, inner]])


def _build_program():
    nc = bass.Bass("TRN2", target_bir_lowering=False, debug=False)

    g = nc.dram_tensor("g", [NS, 2, H, W], F32, kind="ExternalInput")
    pf = nc.dram_tensor("pf", [P, NS * NPF], F32, kind="ExternalInput")
    pi = nc.dram_tensor("pi", [1, 2 * NS], I32, kind="ExternalInput")
    iv = nc.dram_tensor("iv", [P, NS * Q], F16, kind="ExternalInput")
    mats = nc.dram_tensor("mats", [P, NS * 2 * P], F16, kind="ExternalInput")
    xmats = nc.dram_tensor("xmats", [P, NS * 2 * P], F16, kind="ExternalInput")
    out = nc.dram_tensor("out", [NS, W], F32, kind="ExternalOutput")
    RH = 272
    gdl = nc.dram_tensor("gdl", [NS * RH * W, 1], F16)
    gdh = nc.dram_tensor("gdh", [NS * RH * W, 1], F16)

    with tile.TileContext(nc) as tc:
        with (
            tc.tile_pool(name="consts", bufs=1) as consts,
            tc.tile_pool(name="io", bufs=2) as io,
            tc.tile_pool(name="work", bufs=2) as work,
            tc.tile_pool(name="psd", bufs=2, space="PSUM") as psdp,
        ):
            g0_first = io.tile([P, Q, W], F32, tag="g0", name="g0_0")
            nc.sync.dma_start(g0_first[:], _grid_ap(g, 0, 0))
            g1_first = io.tile([P, Q, W], F32, tag="g1", name="g1_0")
            nc.sync.dma_start(g1_first[:], _grid_ap(g, 0, 1))

            pfsb = consts.tile([P, NS * NPF], F32)
            nc.sync.dma_start(pfsb[:], pf[:])
            pisb = consts.tile([1, 2 * NS], I32)
            nc.sync.dma_start(pisb[:], pi[:])
            ivsb = consts.tile([P, NS * Q], F16)
            nc.sync.dma_start(ivsb[:], iv[:])
            matsb = consts.tile([P, NS * 2 * P], F16)
            nc.sync.dma_start(matsb[:], mats[:])
            xmatsb = consts.tile([P, NS * 2 * P], F16)
            nc.sync.dma_start(xmatsb[:], xmats[:])
            osb = consts.tile([1, NS * W], F32)

            zp = consts.tile([P, RPAD * W // P], F16)
            nc.vector.memset(zp[:], 0.0)
            for s in range(NS):
                nc.sync.dma_start(
                    bass.AP(tensor=gdh, offset=s * RH * W + 256 * W,
                            ap=[[RPAD * W // P, P], [1, RPAD * W // P]]),
                    zp[:],
                )

            for s in range(NS):
                pcol = lambda c: pfsb[:, s * NPF + c: s * NPF + c + 1]
                mA = matsb[:, (2 * s) * P:(2 * s + 1) * P]
                mB = matsb[:, (2 * s + 1) * P:(2 * s + 2) * P]
                xA = xmatsb[:, (2 * s) * P:(2 * s + 1) * P]
                xB = xmatsb[:, (2 * s + 1) * P:(2 * s + 2) * P]

                if s == 0:
                    g0sb, g1sb = g0_first, g1_first
                else:
                    g0sb = io.tile([P, Q, W], F32, tag="g0", name=f"g0_{s}")
                    nc.sync.dma_start(g0sb[:], _grid_ap(g, s, 0))
                    g1sb = io.tile([P, Q, W], F32, tag="g1", name=f"g1_{s}")
                    nc.sync.dma_start(g1sb[:], _grid_ap(g, s, 1))

                # G = a*g0 + b*g1 -> fp16, split lo/hi so the lower
                # writeback+window decouple from the upper combine; the
                # 16-row overlap is recomputed into its own tiny tile
                gsb = work.tile([P, Q, W], F16)
                gsbx = work.tile([16, 1, W], F16)
                for (lo_q, hi_q, tag) in ((0, 2, "lo"), (2, 4, "hi")):
                    th0 = work.tile([P, 2, W], F16, tag=f"t0{tag}",
                                    name=f"t0{tag}_{s}")
                    nc.scalar.activation(th0[:], g0sb[:, lo_q:hi_q, :],
                                         mybir.ActivationFunctionType.Copy,
                                         scale=pcol(COL_A))
                    th1 = work.tile([P, 2, W], F16, tag=f"t1{tag}",
                                    name=f"t1{tag}_{s}")
                    nc.vector.tensor_scalar(
                        out=th1[:], in0=g1sb[:, lo_q:hi_q, :],
                        scalar1=pcol(COL_B),
                        scalar2=None, op0=mybir.AluOpType.mult)
                    nc.vector.tensor_tensor(out=gsb[:, lo_q:hi_q, :],
                                            in0=th0[:], in1=th1[:],
                                            op=mybir.AluOpType.add)
                    if tag == "lo":
                        tx0 = work.tile([16, 1, W], F16, tag="tx0",
                                        name=f"tx0_{s}")
                        nc.scalar.activation(
                            tx0[:], g0sb[0:16, 2:3, :],
                            mybir.ActivationFunctionType.Copy,
                            scale=pfsb[0:16, s * NPF + COL_A:
                                       s * NPF + COL_A + 1])
                        tx1 = work.tile([16, 1, W], F16, tag="tx1",
                                        name=f"tx1_{s}")
                        nc.vector.tensor_scalar(
                            out=tx1[:], in0=g1sb[0:16, 2:3, :],
                            scalar1=pfsb[0:16, s * NPF + COL_B:
                                         s * NPF + COL_B + 1],
                            scalar2=None, op0=mybir.AluOpType.mult)
                        nc.vector.tensor_tensor(out=gsbx[:], in0=tx0[:],
                                                in1=tx1[:],
                                                op=mybir.AluOpType.add)
                        nc.sync.dma_start(
                            bass.AP(tensor=gdl, offset=s * RH * W,
                                    ap=[[W, P], [P * W, 2], [1, W]]),
                            gsb[:, 0:2, :])
                        nc.sync.dma_start(
                            bass.AP(tensor=gdl,
                                    offset=s * RH * W + 256 * W,
                                    ap=[[W, 16], [1, W]]),
                            gsbx[:])
                offl = nc.values_load(pisb[0:1, 2 * s: 2 * s + 1],
                                      engines=(mybir.EngineType.SP,),
                                      skip_runtime_bounds_check=True)
                winl = io.tile([P, 2, WLEN], F16)
                nc.sync.dma_start(
                    winl[:], bass.AP(tensor=gdl, offset=offl,
                                     ap=[[W, P], [P * W, 2], [1, WLEN]]))
                nc.sync.dma_start(
                    bass.AP(tensor=gdh, offset=s * RH * W,
                            ap=[[W, P], [P * W, 2], [1, W]]),
                    gsb[:, 2:4, :])
                offh = nc.values_load(pisb[0:1, 2 * s + 1: 2 * s + 2],
                                      engines=(mybir.EngineType.SP,),
                                      skip_runtime_bounds_check=True)
                winh = io.tile([P, 2, WLEN], F16)
                nc.sync.dma_start(
                    winh[:], bass.AP(tensor=gdh, offset=offh,
                                     ap=[[W, P], [P * W, 2], [1, WLEN]]))

                bc = work.tile([P, Q, W], F16)
                for hv, wsrc in ((0, winl), (1, winh)):
                    bch = work.tile([P, 2, W], F16, tag=f"bch{hv}",
                                    name=f"bch{hv}_{s}")
                    nc.vector.tensor_scalar(
                        out=bch[:], in0=wsrc[:, :, 0:W], scalar1=pcol(COL_CB0),
                        scalar2=None, op0=mybir.AluOpType.mult)
                    bc1 = work.tile([P, 2, W], F16, tag=f"bc1{hv}",
                                    name=f"bc1{hv}_{s}")
                    nc.vector.tensor_scalar(
                        out=bc1[:], in0=wsrc[:, :, 1:W + 1],
                        scalar1=pcol(COL_CB1),
                        scalar2=None, op0=mybir.AluOpType.mult)
                    nc.vector.tensor_tensor(out=bc[:, 2 * hv:2 * hv + 2, :],
                                            in0=bch[:], in1=bc1[:],
                                            op=mybir.AluOpType.add)

                # delta (per 128-row block) accumulated in PSUM via PE
                psd = psdp.tile([P, Q, W], F32)
                for q in range(Q):
                    mms = [(mA, gsb[:, q, :]), (mB, bc[:, q, :])]
                    if q < Q - 1:
                        mms += [(xA, gsb[:, q + 1, :]), (xB, bc[:, q + 1, :])]
                    for k, (lhsT, rhs) in enumerate(mms):
                        nc.tensor.matmul(psd[:, q, :], lhsT=lhsT, rhs=rhs,
                                         start=(k == 0), stop=(k == len(mms) - 1))

                # square -> fp16, then ivalid-weighted column sums on PE
                sq = work.tile([P, Q, W], F16)
                nc.scalar.activation(sq[:], psd[:],
                                     mybir.ActivationFunctionType.Square)
                ps = psd[0:1, 0, 0:W]
                for q in range(Q):
                    nc.tensor.matmul(
                        ps, lhsT=ivsb[:, s * Q + q: s * Q + q + 1],
                        rhs=sq[:, q, :], start=(q == 0), stop=(q == Q - 1))
                nc.vector.tensor_copy(osb[0:1, s * W:(s + 1) * W], ps)

            nc.sync.dma_start(out[:], osb[0:1, :])

    return nc


def _host_params(gt_sym_axis, gd_sym_axis):
    B = gt_sym_axis.shape[0]
    gt = gt_sym_axis.astype(np.float32)
    gds = gd_sym_axis.astype(np.float32)
    prm = []
    for i in range(B):
        sx = gds[i, 0]
        sy = gds[i, 1]
        dx = np.float32(-10.0) * gt[i, 0]
        dy = np.float32(10.0) * gt[i, 1]
        dy1f = np.float32(np.floor(dy))
        dx1f = np.float32(np.floor(dx))
        dy1 = int(dy1f)
        dx1 = int(dx1f)
        fy = np.float32(dy - dy1f)
        fx = np.float32(dx - dx1f)
        pos = bool(dx > 0)
        one = np.float32(1.0)
        zero = np.float32(0.0)
        if pos:
            wa = (one, zero)
            wb = (-(one - fy), -fy)
            rb, cb = dy1, dx1
            jlo, jhi = 0, W - dx1 - 1
        else:
            wa = (-fy, -(one - fy))
            wb = (one, zero)
            rb, cb = dy1 + 1, dx1
            jlo, jhi = -dx1, W
        rows = H - dy1 - 1
        cols = (W - dx1 - 1) if pos else (W + dx1)
        m = max(abs(float(sx)), abs(float(sy)), 1e-30)
        a = np.float32(float(sy) / m)
        b = np.float32(float(sx) / m)
        wf = np.array([a, b, one - fx, fx], dtype=np.float32)
        assert 0 <= rb <= RPAD - 4 and -16 <= cb <= 16 and 0 <= jlo <= jhi <= W
        prm.append(dict(wf=wf, wa=wa, wb=wb, rb=rb, cb=cb, jlo=jlo, jhi=jhi,
                        rows=rows, cols=cols, scale=m * m))
    return prm


def _band(w0, w1):
    """lhsT[k, m] = w0*d(k==m) + w1*d(k==m+1)."""
    mat = np.zeros((P, P), np.float16)
    idx = np.arange(P)
    mat[idx, idx] = np.float16(w0)
    mat[idx[1:], idx[:-1]] = np.float16(w1)
    return mat


def _xband(w1):
    """cross-seam lhsT[k, m] = w1*d(k==0, m==127)."""
    mat = np.zeros((P, P), np.float16)
    mat[0, P - 1] = np.float16(w1)
    return mat


def kernel(grid, gt_sym_axis, gd_sym_axis):
    grid = np.ascontiguousarray(grid, dtype=np.float32)
    B = grid.shape[0]
    assert grid.shape == (B, 2, H, W) and B == NS * NCORES

    if "nc" not in _CACHE:
        nc = _build_program()
        _split_multiwaits(nc)
        _CACHE["nc"] = nc
    nc = _CACHE["nc"]

    prm = _host_params(np.asarray(gt_sym_axis), np.asarray(gd_sym_axis))

    i_of_pq = np.arange(H).reshape(Q, P).T
    in_maps = []
    for c in range(NCORES):
        pfv = np.zeros((P, NS * NPF), np.float32)
        piv = np.zeros((1, 2 * NS), np.int32)
        ivv = np.zeros((P, NS * Q), np.float16)
        matv = np.zeros((P, NS * 2 * P), np.float16)
        xmatv = np.zeros((P, NS * 2 * P), np.float16)
        for s in range(NS):
            p = prm[c * NS + s]
            pfv[:, s * NPF:(s + 1) * NPF] = p["wf"][None, :]
            piv[0, 2 * s] = s * 272 * W + p["rb"] * W + p["cb"]
            piv[0, 2 * s + 1] = s * 272 * W + p["rb"] * W + p["cb"]
            ivv[:, s * Q:(s + 1) * Q] = (i_of_pq < p["rows"]).astype(np.float16)
            matv[:, (2 * s) * P:(2 * s + 1) * P] = _band(*p["wa"])
            matv[:, (2 * s + 1) * P:(2 * s + 2) * P] = _band(*p["wb"])
            xmatv[:, (2 * s) * P:(2 * s + 1) * P] = _xband(p["wa"][1])
            xmatv[:, (2 * s + 1) * P:(2 * s + 2) * P] = _xband(p["wb"][1])
        in_maps.append({
            "g": grid[c * NS:(c + 1) * NS],
            "pf": pfv, "pi": piv, "iv": ivv, "mats": matv, "xmats": xmatv,
        })

    res = run_bass_kernel_spmd(nc, in_maps, core_ids=list(range(NCORES)))

    losses = np.zeros(B, np.float64)
    for c in range(NCORES):
        o = res.results[c]["out"]
        for s in range(NS):
            p = prm[c * NS + s]
            ssq = float(o[s, p["jlo"]:p["jhi"]].sum(dtype=np.float64))
            count = float(np.float32(p["rows"] * p["cols"]))
            losses[c * NS + s] = p["scale"] * ssq / count
    return np.float32(losses.mean())

